# revision 2
# baseline (speedup 1.0000x reference)
"""NormLinearAttention TRN2 kernel v3 — fused BC with resident xT.

vs baseline: q/u projections are computed on-chip inside the attention
phase (no DRAM spill/readback of q,u; ~34MB/core less HBM traffic); the
fused phase works 256-token windows with an explicit software pipeline:
q/u stage runs LEAD windows ahead of attention so the kv pair-AllReduce
and the kv clamp chain hide under projection matmuls; startup DMAs are
interleaved (wk_c, wv_c, x_c) so the first rowtile chain starts after
~0.8MB; output is written bf16 (host casts to f32), halving output DMA.
"""

import numpy as np
import ml_dtypes

import concourse.bass as bass
import concourse.mybir as mybir
import concourse.tile as tile
from concourse import bacc
from concourse.bass_utils import run_bass_kernel_spmd

B, N, D, H = 4, 8192, 1024, 16
HD = D // H          # 64
P = 128
DC = D // P          # 8 dim chunks
NCORES = 8
R_FULL = B * N // NCORES   # 4096 rows per core
WIN = 512            # phase A rowtile-group (psum free size)
BWIN = 256           # fused-BC window (tokens)
EPS = 1e-5
GROUPS = [[0, 1], [2, 3], [4, 5], [6, 7]]
PIPE = 2             # windows between LN/z and out-projection
LEAD = 3             # windows of q/u projection lookahead (covers collective)
XQ = 4               # x DMA split (row quarters)

bf16 = mybir.dt.bfloat16
f32 = mybir.dt.float32
AF = mybir.ActivationFunctionType
ALU = mybir.AluOpType
NPBF16 = ml_dtypes.bfloat16


def build(R=R_FULL, rep=1, collective=True):
    RT = R // P          # rowtiles
    NW = R // BWIN       # fused-BC windows
    RPW = BWIN // P      # rowtiles per BC window

    nc = bacc.Bacc("TRN2", target_bir_lowering=False, debug=False,
                   enable_asserts=False, num_devices=NCORES)

    xt_ext = nc.dram_tensor("xt", [DC, P, R], bf16, kind="ExternalInput").ap()
    w_ext = {n: nc.dram_tensor(n, [D, D], bf16, kind="ExternalInput").ap()
             for n in ("wk", "wv", "wq", "wu", "wo")}
    bkr_ext = nc.dram_tensor("bk_row", [1, D], bf16, kind="ExternalInput").ap()
    bvr_ext = nc.dram_tensor("bv_row", [1, D], bf16, kind="ExternalInput").ap()
    bob_ext = nc.dram_tensor("bo_b", [P, D], f32, kind="ExternalInput").ap()
    bqf_ext = nc.dram_tensor("bq_fm", [P, DC], f32, kind="ExternalInput").ap()
    buf_ext = nc.dram_tensor("bu_fm", [P, DC], f32, kind="ExternalInput").ap()
    lnw_ext = nc.dram_tensor("lnw_fm", [P, DC], f32, kind="ExternalInput").ap()
    lnb_ext = nc.dram_tensor("lnb_fm", [P, DC], f32, kind="ExternalInput").ap()
    out_ext = nc.dram_tensor("out", [R, D], bf16, kind="ExternalOutput").ap()

    with tile.TileContext(nc, num_cores=NCORES) as tc:
        with (
            tc.tile_pool(name="const", bufs=1) as cp,
            tc.tile_pool(name="wop", bufs=1) as wop,
            tc.tile_pool(name="wqu", bufs=2) as wqu,
            tc.tile_pool(name="xtp", bufs=1) as xtp,
            tc.tile_pool(name="wps", bufs=6, space="PSUM") as wps,
            tc.tile_pool(name="accps", bufs=1, space="PSUM") as accps,
            tc.tile_pool(name="dram", bufs=1, space="DRAM") as dram,
            tc.tile_pool(name="small", bufs=1) as sp,
        ):
            # ---- constants ----
            ones128 = cp.tile([P, 1], bf16, name="ones128")
            nc.vector.memset(ones128[:], 1.0)
            ones_row = cp.tile([1, P], bf16, name="ones_row")
            nc.vector.memset(ones_row[:], 1.0)

            bk_row = cp.tile([1, D], bf16, name="bk_row")
            nc.sync.dma_start(bk_row[:], bkr_ext)
            bv_row = cp.tile([1, D], bf16, name="bv_row")
            nc.sync.dma_start(bv_row[:], bvr_ext)
            bo_b = cp.tile([P, D], f32, name="bo_b")
            nc.sync.dma_start(bo_b[:], bob_ext)
            bq_fm = cp.tile([P, DC], f32, name="bq_fm")
            nc.sync.dma_start(bq_fm[:], bqf_ext)
            bu_fm = cp.tile([P, DC], f32, name="bu_fm")
            nc.sync.dma_start(bu_fm[:], buf_ext)
            lnw_fm = cp.tile([P, DC], f32, name="lnw_fm")
            nc.sync.dma_start(lnw_fm[:], lnw_ext)
            lnb_fm = cp.tile([P, DC], f32, name="lnb_fm")
            nc.sync.dma_start(lnb_fm[:], lnb_ext)

            w_sb = {}
            for n in ("wq", "wu"):
                w_sb[n] = wqu.tile([P, DC, D], bf16, name=f"{n}_sb", tag="Wqu")
            w_sb["wo"] = wop.tile([P, DC, D], bf16, name="wo_sb")

            for _rep in range(rep):
              kv_in = dram.tile([P, DC * P], f32, name="kv_in")
              kv_out = dram.tile([P, DC * P], f32, name="kv_out")
              kv_blk = sp.tile([P, DC * P], bf16, name="kv_blk", bufs=1)
              kv_sb = sp.tile([P, DC * P], f32, name="kv_sb", bufs=1)
              kv_cl = sp.tile([P, DC * P], bf16, name="kv_cl", bufs=1)

              xT = [xtp.tile([P, R], bf16, name=f"xT{c}", tag=f"xT{c}")
                    for c in range(DC)]

              # ---- phase A (scoped pools; freed before BC) ----
              with (
                  tc.tile_pool(name="wkv", bufs=2) as wkv,
                  tc.tile_pool(name="ab", bufs=2) as ab,
              ):
                  for n in ("wk", "wv"):
                      w_sb[n] = wkv.tile([P, DC, D], bf16, name=f"{n}_sb",
                                         tag="W")

                  # loads in consumption order: (wk_c, wv_c, x_c quarter 0)
                  # triplets so rowtile 0's chain starts almost immediately,
                  # then x quarters 1-3, then wq/wu/wo
                  RQ = R // XQ
                  for c in range(DC):
                      nc.sync.dma_start(w_sb["wk"][:, c, :],
                                        w_ext["wk"][c * P:(c + 1) * P, :])
                      nc.sync.dma_start(w_sb["wv"][:, c, :],
                                        w_ext["wv"][c * P:(c + 1) * P, :])
                      nc.sync.dma_start(xT[c][:, 0:RQ], xt_ext[c, :, 0:RQ])
                  for qr in range(1, XQ):
                      for c in range(DC):
                          nc.sync.dma_start(
                              xT[c][:, qr * RQ:(qr + 1) * RQ],
                              xt_ext[c, :, qr * RQ:(qr + 1) * RQ])
                  for n in ("wq", "wu", "wo"):
                      for c in range(DC):
                          nc.sync.dma_start(w_sb[n][:, c, :],
                                            w_ext[n][c * P:(c + 1) * P, :])

                  kv_ps = accps.tile([P, DC * P], f32, name="kv_ps")
                  for rt in range(RT):
                      pk0 = wps.tile([P, WIN], f32, name="pk0", tag="work")
                      pk1 = wps.tile([P, WIN], f32, name="pk1", tag="work")
                      pv0 = wps.tile([P, WIN], f32, name="pv0", tag="work")
                      pv1 = wps.tile([P, WIN], f32, name="pv1", tag="work")
                      nc.tensor.matmul(pk0[:], ones_row[:], bk_row[:, 0:WIN],
                                       start=True, stop=False)
                      nc.tensor.matmul(pk1[:], ones_row[:], bk_row[:, WIN:D],
                                       start=True, stop=False)
                      nc.tensor.matmul(pv0[:], ones_row[:], bv_row[:, 0:WIN],
                                       start=True, stop=False)
                      nc.tensor.matmul(pv1[:], ones_row[:], bv_row[:, WIN:D],
                                       start=True, stop=False)
                      for c in range(DC):
                          st, sto = False, c == DC - 1
                          lhs = xT[c][:, rt * P:(rt + 1) * P]
                          nc.tensor.matmul(pk0[:], lhs, w_sb["wk"][:, c, 0:WIN],
                                           start=st, stop=sto)
                          nc.tensor.matmul(pk1[:], lhs, w_sb["wk"][:, c, WIN:D],
                                           start=st, stop=sto)
                          nc.tensor.matmul(pv0[:], lhs, w_sb["wv"][:, c, 0:WIN],
                                           start=st, stop=sto)
                          nc.tensor.matmul(pv1[:], lhs, w_sb["wv"][:, c, WIN:D],
                                           start=st, stop=sto)
                      k_bf = ab.tile([P, D], bf16, name="k_bf", tag="kvt", bufs=4)
                      v_bf = ab.tile([P, D], bf16, name="v_bf", tag="kvt", bufs=4)
                      nc.scalar.activation(k_bf[:, 0:WIN], pk0[:], AF.Relu)
                      nc.scalar.activation(k_bf[:, WIN:D], pk1[:], AF.Relu)
                      nc.scalar.activation(v_bf[:, 0:WIN], pv0[:], AF.Copy)
                      nc.scalar.activation(v_bf[:, WIN:D], pv1[:], AF.Copy)
                      for g in range(DC):
                          nc.tensor.matmul(
                              kv_ps[:, g * P:(g + 1) * P],
                              k_bf[:, g * P:(g + 1) * P],
                              v_bf[:, g * P:(g + 1) * P],
                              start=(rt == 0 and g % 4 == 0),
                              stop=(rt == RT - 1 and g % 4 == 3),
                          )

                  nc.vector.tensor_copy(kv_sb[:], kv_ps[:])

              # collective + kv prep (small outer-pool buffers only)
              if collective:
                  nc.sync.dma_start(kv_in[:], kv_sb[:])
                  nc.gpsimd.collective_compute(
                      "AllReduce", ALU.add, replica_groups=GROUPS,
                      ins=[kv_in[:]], outs=[kv_out[:]],
                  )
                  # readback overwrites kv_sb (partial no longer needed)
                  nc.sync.dma_start(kv_sb[:], kv_out[:])

              # clamp to [-100,100], |.| >= 0.01 keeping sign
              nc.vector.tensor_scalar(kv_sb[:], kv_sb[:], -100.0, 100.0,
                                      op0=ALU.max, op1=ALU.min)
              nc.scalar.activation(kv_cl[:], kv_sb[:], AF.Sign)
              nc.scalar.activation(kv_sb[:], kv_sb[:], AF.Abs)
              nc.vector.tensor_scalar(kv_sb[:], kv_sb[:], 0.01, None,
                                      op0=ALU.max)
              nc.vector.tensor_tensor(kv_cl[:], kv_cl[:], kv_sb[:], ALU.mult)
              nc.vector.memset(kv_blk[:], 0.0)
              for g in range(DC):
                  nc.vector.tensor_copy(kv_blk[0:HD, g * P:g * P + HD],
                                        kv_cl[0:HD, g * P:g * P + HD])
                  nc.vector.tensor_copy(kv_blk[HD:P, g * P + HD:(g + 1) * P],
                                        kv_cl[HD:P, g * P + HD:(g + 1) * P])

              # ---- fused phase BC, software pipelined:
              #   iteration i: S(i) q/u-project | A(i-LEAD) attn+LN+z |
              #                O(i-LEAD-PIPE) out-project
              with tc.tile_pool(name="pc", bufs=2) as pc:
                  qw_tiles, uw_tiles, zw_tiles = {}, {}, {}
                  for i in range(NW + LEAD + PIPE):
                    if i < NW:
                      w = i
                      x_lo, x_hi = w * BWIN, (w + 1) * BWIN
                      q_w = pc.tile([P, DC, BWIN], bf16, name="q_w", tag="qw",
                                    bufs=LEAD + 1)
                      u_w = pc.tile([P, DC, BWIN], bf16, name="u_w", tag="uw",
                                    bufs=LEAD + 1)
                      for nm, bias, func, dst in (
                          ("wq", bq_fm, AF.Relu, q_w),
                          ("wu", bu_fm, AF.Identity, u_w),
                      ):
                          for t in range(DC):
                              ps = wps.tile([P, BWIN], f32, name="pqu",
                                            tag="work")
                              for c in range(DC):
                                  nc.tensor.matmul(
                                      ps[:], w_sb[nm][:, c, t * P:(t + 1) * P],
                                      xT[c][:, x_lo:x_hi],
                                      start=(c == 0), stop=(c == DC - 1))
                              nc.scalar.activation(dst[:, t, :], ps[:], func,
                                                   bias=bias[:, t:t + 1],
                                                   scale=1.0)
                      qw_tiles[w] = q_w
                      uw_tiles[w] = u_w

                    if LEAD <= i < NW + LEAD:
                      w = i - LEAD
                      q_w = qw_tiles.pop(w)
                      u_w = uw_tiles[w]
                      attn = pc.tile([P, DC, BWIN], bf16, name="attn",
                                     tag="attn")
                      for g in range(DC):
                          aps = wps.tile([P, BWIN], f32, name="aps", tag="work")
                          nc.tensor.matmul(aps[:], kv_blk[:, g * P:(g + 1) * P],
                                           q_w[:, g, :], start=True, stop=True)
                          nc.scalar.activation(attn[:, g, :], aps[:], AF.Copy)

                      # LN stats: per-column sums over all 1024 dims
                      s_ps = wps.tile([1, BWIN], f32, name="s_ps", tag="work")
                      q_ps = wps.tile([1, BWIN], f32, name="q_ps", tag="work")
                      for g in range(DC):
                          attn2 = pc.tile([P, BWIN], bf16, name="attn2",
                                          tag="attn2", bufs=3)
                          nc.vector.tensor_tensor(attn2[:], attn[:, g, :],
                                                  attn[:, g, :], ALU.mult)
                          nc.tensor.matmul(s_ps[:], ones128[:], attn[:, g, :],
                                           start=(g == 0), stop=(g == DC - 1))
                          nc.tensor.matmul(q_ps[:], ones128[:], attn2[:],
                                           start=(g == 0), stop=(g == DC - 1))
                      mean_t = pc.tile([1, BWIN], f32, name="mean_t",
                                       tag="mean_t")
                      var_t = pc.tile([1, BWIN], f32, name="var_t", tag="var_t")
                      nc.vector.tensor_scalar(mean_t[:], s_ps[:], 1.0 / D, None,
                                              op0=ALU.mult)
                      nc.vector.tensor_tensor(var_t[:], mean_t[:], mean_t[:],
                                              ALU.mult)
                      nc.vector.scalar_tensor_tensor(var_t[:], q_ps[:], 1.0 / D,
                                                     var_t[:], ALU.mult,
                                                     ALU.subtract)
                      nc.vector.tensor_scalar(var_t[:], var_t[:], EPS, None,
                                              op0=ALU.add)
                      nc.vector.reciprocal(var_t[:], var_t[:])
                      rstd = pc.tile([1, BWIN], bf16, name="rstd", tag="rstd")
                      nc.scalar.activation(rstd[:], var_t[:], AF.Sqrt)
                      shp = pc.tile([1, BWIN], bf16, name="shp", tag="shp")
                      nc.vector.scalar_tensor_tensor(shp[:], mean_t[:], -1.0,
                                                     rstd[:], ALU.mult,
                                                     ALU.mult)
                      rstd_b = pc.tile([P, BWIN], bf16, name="rstd_b",
                                       tag="rstd_b")
                      nc.gpsimd.partition_broadcast(rstd_b[:], rstd[:])
                      shp_b = pc.tile([P, BWIN], bf16, name="shp_b",
                                      tag="shp_b")
                      nc.gpsimd.partition_broadcast(shp_b[:], shp[:])

                      # z = ((attn * rstd + shiftpre) * lnw + lnb) * u
                      zw = pc.tile([P, DC, BWIN], bf16, name="zw", tag="zw",
                                   bufs=PIPE + 1)
                      for g in range(DC):
                          zt = pc.tile([P, BWIN], bf16, name="zt", tag="zt",
                                       bufs=3)
                          nc.vector.tensor_tensor(zt[:], attn[:, g, :],
                                                  rstd_b[:], ALU.mult)
                          nc.vector.tensor_tensor(zt[:], zt[:], shp_b[:],
                                                  ALU.add)
                          nc.vector.tensor_scalar(zt[:], zt[:],
                                                  lnw_fm[:, g:g + 1],
                                                  lnb_fm[:, g:g + 1],
                                                  op0=ALU.mult, op1=ALU.add)
                          nc.vector.tensor_tensor(zw[:, g, :], zt[:],
                                                  u_w[:, g, :], ALU.mult)
                      del uw_tiles[w]
                      zw_tiles[w] = zw

                    if i >= LEAD + PIPE:
                      w = i - LEAD - PIPE
                      zw = zw_tiles.pop(w)
                      for j in range(RPW):
                          o0 = wps.tile([P, WIN], f32, name="o0", tag="work")
                          o1 = wps.tile([P, WIN], f32, name="o1", tag="work")
                          for c in range(DC):
                              lhs = zw[:, c, j * P:(j + 1) * P]
                              nc.tensor.matmul(o0[:], lhs,
                                               w_sb["wo"][:, c, 0:WIN],
                                               start=(c == 0), stop=(c == DC - 1))
                              nc.tensor.matmul(o1[:], lhs,
                                               w_sb["wo"][:, c, WIN:D],
                                               start=(c == 0), stop=(c == DC - 1))
                          osb = pc.tile([P, D], bf16, name="osb", tag="osb",
                                        bufs=2)
                          nc.vector.scalar_tensor_tensor(osb[:, 0:WIN], o0[:],
                                                         1.0, bo_b[:, 0:WIN],
                                                         ALU.mult, ALU.add)
                          nc.vector.scalar_tensor_tensor(osb[:, WIN:D], o1[:],
                                                         1.0, bo_b[:, WIN:D],
                                                         ALU.mult, ALU.add)
                          rt = w * RPW + j
                          nc.sync.dma_start(out_ext[rt * P:(rt + 1) * P, :],
                                            osb[:])

    nc.compile()
    return nc


def make_in_maps(query, Wq, bq, Wk, bk, Wv, bv, Wu, bu, Wo, bo, ln_w, ln_b,
                 R=R_FULL):
    xs = query.reshape(-1, D).astype(NPBF16)
    common = {
        "wk": np.ascontiguousarray(Wk).astype(NPBF16),
        "wv": np.ascontiguousarray(Wv).astype(NPBF16),
        "wq": np.ascontiguousarray(Wq).astype(NPBF16),
        "wu": np.ascontiguousarray(Wu).astype(NPBF16),
        "wo": np.ascontiguousarray(Wo).astype(NPBF16),
        "bk_row": np.ascontiguousarray(bk.astype(NPBF16).reshape(1, D)),
        "bv_row": np.ascontiguousarray(bv.astype(NPBF16).reshape(1, D)),
        "bo_b": np.ascontiguousarray(
            np.broadcast_to(bo.astype(np.float32), (P, D))),
        "bq_fm": np.ascontiguousarray(bq.astype(np.float32).reshape(DC, P).T),
        "bu_fm": np.ascontiguousarray(bu.astype(np.float32).reshape(DC, P).T),
        "lnw_fm": np.ascontiguousarray(ln_w.astype(np.float32).reshape(DC, P).T),
        "lnb_fm": np.ascontiguousarray(ln_b.astype(np.float32).reshape(DC, P).T),
    }
    return [dict(common, xt=np.ascontiguousarray(
                xs[c * R:(c + 1) * R].T.reshape(DC, P, R)))
            for c in range(NCORES)]


_NC_CACHE = {}


def kernel(query, Wq, bq, Wk, bk, Wv, bv, Wu, bu, Wo, bo, ln_w, ln_b):
    query = np.asarray(query, dtype=np.float32)
    if "nc" not in _NC_CACHE:
        _NC_CACHE["nc"] = build()
    nc = _NC_CACHE["nc"]
    in_maps = make_in_maps(query, np.asarray(Wq), np.asarray(bq),
                           np.asarray(Wk), np.asarray(bk),
                           np.asarray(Wv), np.asarray(bv),
                           np.asarray(Wu), np.asarray(bu),
                           np.asarray(Wo), np.asarray(bo),
                           np.asarray(ln_w), np.asarray(ln_b))
    res = run_bass_kernel_spmd(nc, in_maps, list(range(NCORES)))
    out = np.empty((B * N, D), np.float32)
    for c in range(NCORES):
        out[c * R_FULL:(c + 1) * R_FULL] = res.results[c]["out"].astype(
            np.float32)
    return out.reshape(B, N, D)


# revision 3
# speedup vs baseline: 1.0280x; 1.0280x over previous
"""NormLinearAttention TRN2 kernel v3 — fused BC with resident xT.

vs baseline: q/u projections are computed on-chip inside the attention
phase (no DRAM spill/readback of q,u; ~34MB/core less HBM traffic); the
fused phase works 256-token windows with an explicit software pipeline:
q/u stage runs LEAD windows ahead of attention so the kv pair-AllReduce
and the kv clamp chain hide under projection matmuls; startup DMAs are
interleaved (wk_c, wv_c, x_c) so the first rowtile chain starts after
~0.8MB; output is written bf16 (host casts to f32), halving output DMA.
"""

import numpy as np
import ml_dtypes

import concourse.bass as bass
import concourse.mybir as mybir
import concourse.tile as tile
from concourse import bacc
from concourse.bass_utils import run_bass_kernel_spmd

B, N, D, H = 4, 8192, 1024, 16
HD = D // H          # 64
P = 128
DC = D // P          # 8 dim chunks
NCORES = 8
R_FULL = B * N // NCORES   # 4096 rows per core
WIN = 512            # phase A rowtile-group (psum free size)
BWIN = 256           # fused-BC window (tokens)
EPS = 1e-5
GROUPS = [[0, 1], [2, 3], [4, 5], [6, 7]]
PIPE = 2             # windows between LN/z and out-projection
LEAD = 3             # windows of q/u projection lookahead (covers collective)
XQ = 4               # x DMA split (row quarters)

bf16 = mybir.dt.bfloat16
f32 = mybir.dt.float32
AF = mybir.ActivationFunctionType
ALU = mybir.AluOpType
NPBF16 = ml_dtypes.bfloat16


def build(R=R_FULL, rep=1, collective=True):
    RT = R // P          # rowtiles
    NW = R // BWIN       # fused-BC windows
    RPW = BWIN // P      # rowtiles per BC window

    nc = bacc.Bacc("TRN2", target_bir_lowering=False, debug=False,
                   enable_asserts=False, num_devices=NCORES)

    xt_ext = nc.dram_tensor("xt", [DC, P, R], bf16, kind="ExternalInput").ap()
    w_ext = {n: nc.dram_tensor(n, [D, D], bf16, kind="ExternalInput").ap()
             for n in ("wk", "wv", "wq", "wu", "wo")}
    bkr_ext = nc.dram_tensor("bk_row", [1, D], bf16, kind="ExternalInput").ap()
    bvr_ext = nc.dram_tensor("bv_row", [1, D], bf16, kind="ExternalInput").ap()
    bob_ext = nc.dram_tensor("bo_b", [P, D], bf16, kind="ExternalInput").ap()
    bqf_ext = nc.dram_tensor("bq_fm", [P, DC], f32, kind="ExternalInput").ap()
    buf_ext = nc.dram_tensor("bu_fm", [P, DC], f32, kind="ExternalInput").ap()
    lnw_ext = nc.dram_tensor("lnw_fm", [P, DC], f32, kind="ExternalInput").ap()
    lnb_ext = nc.dram_tensor("lnb_fm", [P, DC], f32, kind="ExternalInput").ap()
    out_ext = nc.dram_tensor("out", [R, D], bf16, kind="ExternalOutput").ap()

    with tile.TileContext(nc, num_cores=NCORES) as tc:
        with (
            tc.tile_pool(name="const", bufs=1) as cp,
            tc.tile_pool(name="wop", bufs=1) as wop,
            tc.tile_pool(name="wqu", bufs=2) as wqu,
            tc.tile_pool(name="xtp", bufs=1) as xtp,
            tc.tile_pool(name="wps", bufs=6, space="PSUM") as wps,
            tc.tile_pool(name="accps", bufs=1, space="PSUM") as accps,
            tc.tile_pool(name="dram", bufs=1, space="DRAM") as dram,
            tc.tile_pool(name="small", bufs=1) as sp,
        ):
            # ---- constants ----
            ones128 = cp.tile([P, 1], bf16, name="ones128")
            nc.vector.memset(ones128[:], 1.0)
            ones_row = cp.tile([1, P], bf16, name="ones_row")
            nc.vector.memset(ones_row[:], 1.0)

            bk_row = cp.tile([1, D], bf16, name="bk_row")
            nc.sync.dma_start(bk_row[:], bkr_ext)
            bv_row = cp.tile([1, D], bf16, name="bv_row")
            nc.sync.dma_start(bv_row[:], bvr_ext)
            bk_bc = cp.tile([P, D], bf16, name="bk_bc")
            nc.gpsimd.partition_broadcast(bk_bc[:], bk_row[:])
            bv_bc = cp.tile([P, D], bf16, name="bv_bc")
            nc.gpsimd.partition_broadcast(bv_bc[:], bv_row[:])
            bo_b = cp.tile([P, D], bf16, name="bo_b")
            nc.sync.dma_start(bo_b[:], bob_ext)
            bq_fm = cp.tile([P, DC], f32, name="bq_fm")
            nc.sync.dma_start(bq_fm[:], bqf_ext)
            bu_fm = cp.tile([P, DC], f32, name="bu_fm")
            nc.sync.dma_start(bu_fm[:], buf_ext)
            lnw_fm = cp.tile([P, DC], f32, name="lnw_fm")
            nc.sync.dma_start(lnw_fm[:], lnw_ext)
            lnb_fm = cp.tile([P, DC], f32, name="lnb_fm")
            nc.sync.dma_start(lnb_fm[:], lnb_ext)

            w_sb = {}
            for n in ("wq", "wu"):
                w_sb[n] = wqu.tile([P, DC, D], bf16, name=f"{n}_sb", tag="Wqu")
            w_sb["wo"] = wop.tile([P, DC, D], bf16, name="wo_sb")

            for _rep in range(rep):
              kv_in = dram.tile([P, DC * P], f32, name="kv_in")
              kv_out = dram.tile([P, DC * P], f32, name="kv_out")
              kv_blk = sp.tile([P, DC * P], bf16, name="kv_blk", bufs=1)
              kv_sb = sp.tile([P, DC * P], f32, name="kv_sb", bufs=1)
              kv_cl = sp.tile([P, DC * P], bf16, name="kv_cl", bufs=1)

              xT = [xtp.tile([P, R], bf16, name=f"xT{c}", tag=f"xT{c}")
                    for c in range(DC)]

              # ---- phase A (scoped pools; freed before BC) ----
              with (
                  tc.tile_pool(name="wkv", bufs=2) as wkv,
                  tc.tile_pool(name="ab", bufs=2) as ab,
              ):
                  for n in ("wk", "wv"):
                      w_sb[n] = wkv.tile([P, DC, D], bf16, name=f"{n}_sb",
                                         tag="W")

                  # loads in consumption order: (wk_c, wv_c, x_c quarter 0)
                  # triplets so rowtile 0's chain starts almost immediately,
                  # then x quarters 1-3, then wq/wu/wo
                  RQ = R // XQ
                  for c in range(DC):
                      nc.sync.dma_start(w_sb["wk"][:, c, :],
                                        w_ext["wk"][c * P:(c + 1) * P, :])
                      nc.sync.dma_start(w_sb["wv"][:, c, :],
                                        w_ext["wv"][c * P:(c + 1) * P, :])
                      nc.sync.dma_start(xT[c][:, 0:RQ], xt_ext[c, :, 0:RQ])
                  for qr in range(1, XQ):
                      for c in range(DC):
                          nc.sync.dma_start(
                              xT[c][:, qr * RQ:(qr + 1) * RQ],
                              xt_ext[c, :, qr * RQ:(qr + 1) * RQ])
                  for n in ("wq", "wu", "wo"):
                      for c in range(DC):
                          nc.sync.dma_start(w_sb[n][:, c, :],
                                            w_ext[n][c * P:(c + 1) * P, :])

                  kv_ps = accps.tile([P, DC * P], f32, name="kv_ps")
                  for rt in range(RT):
                      pk0 = wps.tile([P, WIN], f32, name="pk0", tag="work")
                      pk1 = wps.tile([P, WIN], f32, name="pk1", tag="work")
                      pv0 = wps.tile([P, WIN], f32, name="pv0", tag="work")
                      pv1 = wps.tile([P, WIN], f32, name="pv1", tag="work")
                      for c in range(DC):
                          st, sto = c == 0, c == DC - 1
                          lhs = xT[c][:, rt * P:(rt + 1) * P]
                          nc.tensor.matmul(pk0[:], lhs, w_sb["wk"][:, c, 0:WIN],
                                           start=st, stop=sto)
                          nc.tensor.matmul(pk1[:], lhs, w_sb["wk"][:, c, WIN:D],
                                           start=st, stop=sto)
                          nc.tensor.matmul(pv0[:], lhs, w_sb["wv"][:, c, 0:WIN],
                                           start=st, stop=sto)
                          nc.tensor.matmul(pv1[:], lhs, w_sb["wv"][:, c, WIN:D],
                                           start=st, stop=sto)
                      k_bf = ab.tile([P, D], bf16, name="k_bf", tag="kvt", bufs=4)
                      v_bf = ab.tile([P, D], bf16, name="v_bf", tag="kvt", bufs=4)
                      kt = ab.tile([P, D], f32, name="kt", tag="ktmp", bufs=2)
                      nc.vector.tensor_tensor(kt[:, 0:WIN], pk0[:],
                                              bk_bc[:, 0:WIN], ALU.add)
                      nc.vector.tensor_tensor(kt[:, WIN:D], pk1[:],
                                              bk_bc[:, WIN:D], ALU.add)
                      nc.scalar.activation(k_bf[:, 0:WIN], kt[:, 0:WIN], AF.Relu)
                      nc.scalar.activation(k_bf[:, WIN:D], kt[:, WIN:D], AF.Relu)
                      nc.vector.tensor_tensor(v_bf[:, 0:WIN], pv0[:],
                                              bv_bc[:, 0:WIN], ALU.add)
                      nc.vector.tensor_tensor(v_bf[:, WIN:D], pv1[:],
                                              bv_bc[:, WIN:D], ALU.add)
                      for g in range(DC):
                          nc.tensor.matmul(
                              kv_ps[:, g * P:(g + 1) * P],
                              k_bf[:, g * P:(g + 1) * P],
                              v_bf[:, g * P:(g + 1) * P],
                              start=(rt == 0 and g % 4 == 0),
                              stop=(rt == RT - 1 and g % 4 == 3),
                          )

                  nc.vector.tensor_copy(kv_sb[:], kv_ps[:])

              # collective + kv prep (small outer-pool buffers only)
              if collective:
                  nc.sync.dma_start(kv_in[:], kv_sb[:])
                  nc.gpsimd.collective_compute(
                      "AllReduce", ALU.add, replica_groups=GROUPS,
                      ins=[kv_in[:]], outs=[kv_out[:]],
                  )
                  # readback overwrites kv_sb (partial no longer needed)
                  nc.sync.dma_start(kv_sb[:], kv_out[:])

              # clamp to [-100,100], |.| >= 0.01 keeping sign -- pipelined
              # per half/per g so attention g=0 unblocks early
              nc.vector.memset(kv_blk[:], 0.0)
              for h in range(2):
                  hs = slice(h * (DC * P // 2), (h + 1) * (DC * P // 2))
                  nc.vector.tensor_scalar(kv_sb[:, hs], kv_sb[:, hs],
                                          -100.0, 100.0,
                                          op0=ALU.max, op1=ALU.min)
                  nc.scalar.activation(kv_cl[:, hs], kv_sb[:, hs], AF.Sign)
                  nc.scalar.activation(kv_sb[:, hs], kv_sb[:, hs], AF.Abs)
                  nc.vector.tensor_scalar(kv_sb[:, hs], kv_sb[:, hs],
                                          0.01, None, op0=ALU.max)
                  for g in range(h * (DC // 2), (h + 1) * (DC // 2)):
                      gs = slice(g * P, (g + 1) * P)
                      nc.vector.tensor_tensor(kv_cl[:, gs], kv_cl[:, gs],
                                              kv_sb[:, gs], ALU.mult)
                      nc.vector.tensor_copy(kv_blk[0:HD, g * P:g * P + HD],
                                            kv_cl[0:HD, g * P:g * P + HD])
                      nc.vector.tensor_copy(
                          kv_blk[HD:P, g * P + HD:(g + 1) * P],
                          kv_cl[HD:P, g * P + HD:(g + 1) * P])

              # ---- fused phase BC, software pipelined:
              #   iteration i: S(i) q/u-project | A(i-LEAD) attn+LN+z |
              #                O(i-LEAD-PIPE) out-project
              with tc.tile_pool(name="pc", bufs=2) as pc:
                  qw_tiles, uw_tiles, zw_tiles = {}, {}, {}
                  for i in range(NW + LEAD + PIPE):
                    if i < NW:
                      w = i
                      x_lo, x_hi = w * BWIN, (w + 1) * BWIN
                      q_w = pc.tile([P, DC, BWIN], bf16, name="q_w", tag="qw",
                                    bufs=LEAD + 1)
                      u_w = pc.tile([P, DC, BWIN], bf16, name="u_w", tag="uw",
                                    bufs=LEAD + 1)
                      for nm, bias, func, dst in (
                          ("wq", bq_fm, AF.Relu, q_w),
                          ("wu", bu_fm, AF.Identity, u_w),
                      ):
                          for t in range(DC):
                              ps = wps.tile([P, BWIN], f32, name="pqu",
                                            tag="work")
                              for c in range(DC):
                                  nc.tensor.matmul(
                                      ps[:], w_sb[nm][:, c, t * P:(t + 1) * P],
                                      xT[c][:, x_lo:x_hi],
                                      start=(c == 0), stop=(c == DC - 1))
                              nc.scalar.activation(dst[:, t, :], ps[:], func,
                                                   bias=bias[:, t:t + 1],
                                                   scale=1.0)
                      qw_tiles[w] = q_w
                      uw_tiles[w] = u_w

                    if LEAD <= i < NW + LEAD:
                      w = i - LEAD
                      q_w = qw_tiles.pop(w)
                      u_w = uw_tiles[w]
                      attn = pc.tile([P, DC, BWIN], bf16, name="attn",
                                     tag="attn", bufs=3)
                      for g in range(DC):
                          aps = wps.tile([P, BWIN], f32, name="aps", tag="work")
                          nc.tensor.matmul(aps[:], kv_blk[:, g * P:(g + 1) * P],
                                           q_w[:, g, :], start=True, stop=True)
                          nc.scalar.activation(attn[:, g, :], aps[:], AF.Copy)

                      # LN stats. Steady state: DVE partial sums + gpsimd
                      # partition all-reduce (keeps PE free while S-stage work
                      # exists). Drain windows: PE ones-matmul reduction (DVE
                      # is the drain bottleneck, PE has slack there).
                      import concourse.bass_isa as bass_isa
                      drain = w >= NW - LEAD - 2
                      s_b = pc.tile([P, BWIN], f32, name="s_b", tag="s_b",
                                    bufs=1)
                      q_b = pc.tile([P, BWIN], f32, name="q_b", tag="q_b",
                                    bufs=1)
                      if not drain:
                          s_pt = pc.tile([P, BWIN], f32, name="s_pt",
                                         tag="s_pt", bufs=2)
                          q_pt = pc.tile([P, BWIN], f32, name="q_pt",
                                         tag="q_pt", bufs=2)
                          sq = pc.tile([P, BWIN], bf16, name="sq",
                                       tag="sq", bufs=2)
                          nc.vector.tensor_tensor(s_pt[:], attn[:, 0, :],
                                                  attn[:, 1, :], ALU.add)
                          nc.scalar.activation(q_pt[:], attn[:, 0, :],
                                               AF.Square)
                          for g in range(1, DC):
                              nc.scalar.activation(sq[:], attn[:, g, :],
                                                   AF.Square)
                              nc.vector.tensor_tensor(q_pt[:], q_pt[:], sq[:],
                                                      ALU.add)
                              if g >= 2:
                                  nc.vector.tensor_tensor(s_pt[:], s_pt[:],
                                                          attn[:, g, :],
                                                          ALU.add)
                          nc.gpsimd.partition_all_reduce(
                              s_b[:], s_pt[:], P, bass_isa.ReduceOp.add)
                          nc.gpsimd.partition_all_reduce(
                              q_b[:], q_pt[:], P, bass_isa.ReduceOp.add)
                      else:
                          s_ps = wps.tile([1, BWIN], f32, name="s_ps",
                                          tag="work")
                          q_ps = wps.tile([1, BWIN], f32, name="q_ps",
                                          tag="work")
                          for g in range(DC):
                              attn2 = pc.tile([P, BWIN], bf16, name="attn2",
                                              tag="attn2", bufs=3)
                              nc.scalar.activation(attn2[:], attn[:, g, :],
                                                   AF.Square)
                              nc.tensor.matmul(s_ps[:], ones128[:],
                                               attn[:, g, :],
                                               start=(g == 0),
                                               stop=(g == DC - 1))
                              nc.tensor.matmul(q_ps[:], ones128[:], attn2[:],
                                               start=(g == 0),
                                               stop=(g == DC - 1))
                          s_row = pc.tile([1, BWIN], f32, name="s_row",
                                          tag="s_row", bufs=1)
                          q_row = pc.tile([1, BWIN], f32, name="q_row",
                                          tag="q_row", bufs=1)
                          nc.vector.tensor_copy(s_row[:], s_ps[:])
                          nc.vector.tensor_copy(q_row[:], q_ps[:])
                          nc.gpsimd.partition_broadcast(s_b[:], s_row[:])
                          nc.gpsimd.partition_broadcast(q_b[:], q_row[:])
                      # rstd_b = 1/sqrt(var+eps); shp_b = -mean*rstd (all [P,BWIN])
                      mean_b = pc.tile([P, BWIN], f32, name="mean_b",
                                       tag="mean_b", bufs=1)
                      nc.vector.tensor_scalar(mean_b[:], s_b[:], 1.0 / D, None,
                                              op0=ALU.mult)
                      var_b = pc.tile([P, BWIN], f32, name="var_b",
                                      tag="var_b", bufs=1)
                      nc.vector.tensor_tensor(var_b[:], mean_b[:], mean_b[:],
                                              ALU.mult)
                      nc.vector.scalar_tensor_tensor(var_b[:], q_b[:], 1.0 / D,
                                                     var_b[:], ALU.mult,
                                                     ALU.subtract)
                      nc.vector.tensor_scalar(var_b[:], var_b[:], EPS, None,
                                              op0=ALU.add)
                      nc.vector.reciprocal(var_b[:], var_b[:])
                      rstd_b = pc.tile([P, BWIN], bf16, name="rstd_b",
                                       tag="rstd_b")
                      nc.scalar.activation(rstd_b[:], var_b[:], AF.Sqrt)
                      shp_b = pc.tile([P, BWIN], bf16, name="shp_b",
                                      tag="shp_b")
                      nc.vector.scalar_tensor_tensor(shp_b[:], mean_b[:], -1.0,
                                                     rstd_b[:], ALU.mult,
                                                     ALU.mult)

                      # z = ((attn * rstd + shiftpre) * lnw + lnb) * u
                      zw = pc.tile([P, DC, BWIN], bf16, name="zw", tag="zw",
                                   bufs=PIPE + 1)
                      for g in range(DC):
                          zt = pc.tile([P, BWIN], bf16, name="zt", tag="zt",
                                       bufs=2)
                          nc.vector.tensor_tensor(zt[:], attn[:, g, :],
                                                  rstd_b[:], ALU.mult)
                          nc.vector.tensor_tensor(zt[:], zt[:], shp_b[:],
                                                  ALU.add)
                          nc.vector.tensor_scalar(zt[:], zt[:],
                                                  lnw_fm[:, g:g + 1],
                                                  lnb_fm[:, g:g + 1],
                                                  op0=ALU.mult, op1=ALU.add)
                          nc.vector.tensor_tensor(zw[:, g, :], zt[:],
                                                  u_w[:, g, :], ALU.mult)
                      del uw_tiles[w]
                      zw_tiles[w] = zw

                    if i >= LEAD + PIPE:
                      w = i - LEAD - PIPE
                      zw = zw_tiles.pop(w)
                      for j in range(RPW):
                          o0 = wps.tile([P, WIN], f32, name="o0", tag="work")
                          o1 = wps.tile([P, WIN], f32, name="o1", tag="work")
                          for c in range(DC):
                              lhs = zw[:, c, j * P:(j + 1) * P]
                              nc.tensor.matmul(o0[:], lhs,
                                               w_sb["wo"][:, c, 0:WIN],
                                               start=(c == 0), stop=(c == DC - 1))
                              nc.tensor.matmul(o1[:], lhs,
                                               w_sb["wo"][:, c, WIN:D],
                                               start=(c == 0), stop=(c == DC - 1))
                          osb = pc.tile([P, D], bf16, name="osb", tag="osb",
                                        bufs=2)
                          nc.vector.scalar_tensor_tensor(osb[:, 0:WIN], o0[:],
                                                         1.0, bo_b[:, 0:WIN],
                                                         ALU.mult, ALU.add)
                          nc.vector.scalar_tensor_tensor(osb[:, WIN:D], o1[:],
                                                         1.0, bo_b[:, WIN:D],
                                                         ALU.mult, ALU.add)
                          rt = w * RPW + j
                          nc.sync.dma_start(out_ext[rt * P:(rt + 1) * P, :],
                                            osb[:])

    nc.compile()
    return nc


def make_in_maps(query, Wq, bq, Wk, bk, Wv, bv, Wu, bu, Wo, bo, ln_w, ln_b,
                 R=R_FULL):
    xs = query.reshape(-1, D).astype(NPBF16)
    common = {
        "wk": np.ascontiguousarray(Wk).astype(NPBF16),
        "wv": np.ascontiguousarray(Wv).astype(NPBF16),
        "wq": np.ascontiguousarray(Wq).astype(NPBF16),
        "wu": np.ascontiguousarray(Wu).astype(NPBF16),
        "wo": np.ascontiguousarray(Wo).astype(NPBF16),
        "bk_row": np.ascontiguousarray(bk.astype(NPBF16).reshape(1, D)),
        "bv_row": np.ascontiguousarray(bv.astype(NPBF16).reshape(1, D)),
        "bo_b": np.ascontiguousarray(
            np.broadcast_to(bo.astype(NPBF16), (P, D))),
        "bq_fm": np.ascontiguousarray(bq.astype(np.float32).reshape(DC, P).T),
        "bu_fm": np.ascontiguousarray(bu.astype(np.float32).reshape(DC, P).T),
        "lnw_fm": np.ascontiguousarray(ln_w.astype(np.float32).reshape(DC, P).T),
        "lnb_fm": np.ascontiguousarray(ln_b.astype(np.float32).reshape(DC, P).T),
    }
    return [dict(common, xt=np.ascontiguousarray(
                xs[c * R:(c + 1) * R].T.reshape(DC, P, R)))
            for c in range(NCORES)]


_NC_CACHE = {}


def kernel(query, Wq, bq, Wk, bk, Wv, bv, Wu, bu, Wo, bo, ln_w, ln_b):
    query = np.asarray(query, dtype=np.float32)
    if "nc" not in _NC_CACHE:
        _NC_CACHE["nc"] = build()
    nc = _NC_CACHE["nc"]
    in_maps = make_in_maps(query, np.asarray(Wq), np.asarray(bq),
                           np.asarray(Wk), np.asarray(bk),
                           np.asarray(Wv), np.asarray(bv),
                           np.asarray(Wu), np.asarray(bu),
                           np.asarray(Wo), np.asarray(bo),
                           np.asarray(ln_w), np.asarray(ln_b))
    res = run_bass_kernel_spmd(nc, in_maps, list(range(NCORES)))
    out = np.empty((B * N, D), np.float32)
    for c in range(NCORES):
        out[c * R_FULL:(c + 1) * R_FULL] = res.results[c]["out"].astype(
            np.float32)
    return out.reshape(B, N, D)
